# revision 26
# baseline (speedup 1.0000x reference)
"""Grouped-Query Attention forward pass on 8 Trainium2 NeuronCores.

Sharding: 2-way data parallel over batch x 4-way tensor parallel over KV
head groups. Core c = 4*b + g handles batch b and KV group g (4 query
heads + 1 KV head). Each core computes a partial o-projection output
(its head group's contribution, full [S, D]); the host sums the 4
partials per batch.

All matmul operands are fp16 (full PE rate, fp32 PSUM accumulation).

Device kernel per core:
  Phase 1 (per 128-row seq tile): q/k/v projections (contraction over D
    via xT tiles loaded 4 seq-tiles at a time for long DMA lines), RMS
    stats via tensor_tensor_reduce (DVE, eps seeded into the reduction),
    one Sqrt (ACT) + reciprocal (DVE) for all 5 head chunks, fused
    (src*rms)*cos|sin via scalar_tensor_tensor on DVE, rope add on
    GpSimd, PE transpose (4 q-head transposes share one PSUM bank, one
    batched eviction) -> qT/kT fp16. v stays natural.
  Phase 2+3 fused (qc-outer, head-inner): per 512-query chunk and head,
    16 key tiles in 8 two-bank PSUM batches: scoresT = kT.T @ qT -> one
    exp per batch (ACT, fp16 out) -> AV accumulation (lagged 2 batches).
    Softmax denominator: DVE add chain over exp tiles, one gpsimd
    partition_all_reduce (sum over key partitions, broadcast to all
    partitions), reciprocal_approx_fast + normalize on DVE.
    o-projection matmuls for the previous query chunk are interleaved as
    PE filler (skipping the first head so the last head's normalize has
    landed); PSUM evictions rotate over DVE/ACT; one output DMA per seq
    tile.
"""

import sys

sys.path.insert(0, "/opt/trn_rl_repo")

from collections import deque

import numpy as np

import concourse.bass as bass
import concourse.tile as tile
from concourse import bacc, bass_isa, mybir
from concourse.bass_utils import run_bass_kernel_spmd
from concourse.masks import make_identity

F32 = mybir.dt.float32
F32R = mybir.dt.float32r
F16 = mybir.dt.float16
AF = mybir.ActivationFunctionType
MUL = mybir.AluOpType.mult
ADD = mybir.AluOpType.add

B = 2
S = 2048
D = 2048
NH = 16
NKV = 4
HD = 128
G = NH // NKV  # 4 query heads per KV head / per core
DQ = G * HD  # 512 query dims per core
EPS = 1e-6
ROPE_BASE = 10000.0

NT = S // 128  # 16 sequence tiles
ND = D // 128  # 16 contraction slices
QC = 4  # query chunks of 512
KT = S // 128  # 16 key tiles
NB = KT // 2  # 8 two-tile score batches per (head, qc)
XG = 4  # seq tiles per x-column DMA group

_cached_nc = None
last_results = None  # BassKernelResults of the most recent run (for test.py)


def _build_program():
    nc = bacc.Bacc("TRN2", target_bir_lowering=False, debug=False)

    xt = nc.dram_tensor("xt", [D, S], F16, kind="ExternalInput").ap()
    wq = nc.dram_tensor("wq", [D, DQ], F16, kind="ExternalInput").ap()
    wkv = nc.dram_tensor("wkv", [D, 2 * HD], F16, kind="ExternalInput").ap()
    wo = nc.dram_tensor("wo", [DQ, D], F16, kind="ExternalInput").ap()
    cq = nc.dram_tensor("cq", [S, HD], F16, kind="ExternalInput").ap()
    sq = nc.dram_tensor("sq", [S, HD], F16, kind="ExternalInput").ap()
    ck = nc.dram_tensor("ck", [S, HD], F16, kind="ExternalInput").ap()
    sk = nc.dram_tensor("sk", [S, HD], F16, kind="ExternalInput").ap()
    o = nc.dram_tensor("o", [S, D], F32, kind="ExternalOutput").ap()

    def dram3(t, pstep, bstep, nb, line):
        # [128 partitions, nb, line] view of a DRAM matrix
        return lambda off: bass.AP(
            tensor=t.tensor, offset=off, ap=[[pstep, 128], [bstep, nb], [1, line]]
        )

    with tile.TileContext(nc) as tc:
        from contextlib import ExitStack

        with ExitStack() as ctx:
            persist = ctx.enter_context(tc.tile_pool(name="persist", bufs=1))

            # persistent SBUF tensors (all fp16)
            qt_all = persist.tile([128, G, S], F16, tag="qt")
            kt_sb = persist.tile([128, S], F16, tag="kt")
            v_sb = persist.tile([128, KT, HD], F16, tag="v")
            # separate tiles per (head, query-chunk) so the o-projection's
            # reads never alias the normalize writes of other chunks
            outt = [
                [
                    persist.tile([128, 512], F16, name=f"outt{h}_{q}", tag=f"outt{h}_{q}")
                    for q in range(QC)
                ]
                for h in range(G)
            ]
            ident = persist.tile([128, 128], F32, tag="ident")
            make_identity(nc, ident[:])
            eps_sb = persist.tile([128, 1], F32, tag="eps")
            nc.vector.memset(eps_sb[:], EPS)

            # ---------------- Phase 1: projections + RMS + RoPE ----------------
            with ExitStack() as p1:
                p1.enter_context(nc.named_scope("p1_proj"))
                wpool = p1.enter_context(tc.tile_pool(name="w1", bufs=1))
                xpool = p1.enter_context(tc.tile_pool(name="xcol", bufs=2))
                rope = p1.enter_context(tc.tile_pool(name="rope", bufs=4))
                small = p1.enter_context(tc.tile_pool(name="small", bufs=4))
                ps1 = p1.enter_context(tc.tile_pool(name="ps1", bufs=2, space="PSUM"))
                pst = p1.enter_context(tc.tile_pool(name="pst", bufs=2, space="PSUM"))

                wq_sb = wpool.tile([128, ND, DQ], F16, tag="wq")
                wkv_sb = wpool.tile([128, ND, 2 * HD], F16, tag="wkv")
                # first x-column group before everything else, split across 8
                # queues, so the first projection matmul starts ASAP
                xcol0 = xpool.tile([128, ND, XG * 128], F16, name="xcol0", tag="xcol")
                for i in range(8):
                    nc.sync.dma_start(
                        xcol0[:, 2 * i : 2 * i + 2, :],
                        dram3(xt, S, 128 * S, 2, XG * 128)(2 * i * 128 * S),
                    )
                # per-ds-slice weight DMAs: small first transfers so the first
                # projection matmuls are not gated on one monolithic load
                for i in range(ND):
                    nc.sync.dma_start(
                        wq_sb[:, i : i + 1, :],
                        dram3(wq, DQ, 128 * DQ, 1, DQ)(i * 128 * DQ),
                    )
                for i in range(4):
                    nc.sync.dma_start(
                        wkv_sb[:, 4 * i : 4 * i + 4, :],
                        dram3(wkv, 2 * HD, 128 * 2 * HD, 4, 2 * HD)(4 * i * 128 * 2 * HD),
                    )

                cq_sb = wpool.tile([128, NT, HD], F16, tag="cq")
                sq_sb = wpool.tile([128, NT, HD], F16, tag="sq")
                ck_sb = wpool.tile([128, NT, HD], F16, tag="ck")
                sk_sb = wpool.tile([128, NT, HD], F16, tag="sk")
                for t, t_sb in ((cq, cq_sb), (sq, sq_sb), (ck, ck_sb), (sk, sk_sb)):
                    nc.sync.dma_start(t_sb[:], dram3(t, HD, 128 * HD, NT, HD)(0))

                xcol = None
                for st in range(NT):
                    if st == 0:
                        xcol = xcol0
                    elif st % XG == 0:
                        xcol = xpool.tile([128, ND, XG * 128], F16, name="xcol", tag="xcol")
                        # xt[d, s] slice s in [st*128, +XG*128): partition d%128
                        for i in range(4):
                            nc.sync.dma_start(
                                xcol[:, 4 * i : 4 * i + 4, :],
                                dram3(xt, S, 128 * S, 4, XG * 128)(
                                    4 * i * 128 * S + st * 128
                                ),
                            )
                    xsl = slice((st % XG) * 128, (st % XG) * 128 + 128)

                    q_ps = ps1.tile([128, DQ], F32, tag="q_ps")
                    kv_ps = ps1.tile([128, 2 * HD], F32, tag="kv_ps")
                    for ds in range(ND):
                        nc.tensor.matmul(
                            q_ps[:],
                            xcol[:, ds, xsl],
                            wq_sb[:, ds, :],
                            start=(ds == 0),
                            stop=(ds == ND - 1),
                        )
                    for ds in range(ND):
                        nc.tensor.matmul(
                            kv_ps[:],
                            xcol[:, ds, xsl],
                            wkv_sb[:, ds, :],
                            start=(ds == 0),
                            stop=(ds == ND - 1),
                        )

                    # v: straight copy to natural fp16 layout
                    nc.scalar.copy(v_sb[:, st, :], kv_ps[:, HD : 2 * HD])

                    # RMS stats for all 5 head chunks: one batched Square for
                    # the q heads (per-head reduce on DVE), Square+accum for k,
                    # then rms5 = 1/sqrt(sum/HD + eps) for all 5 at once.
                    ssq5 = small.tile([128, G + 1], F32, tag="ssq5")
                    sq_full = small.tile([128, DQ], F16, tag="sq_full")
                    sq_k = small.tile([128, HD], F16, tag="sq_k")
                    nc.scalar.activation(sq_full[:], q_ps[:], AF.Square)
                    nc.vector.tensor_reduce(
                        ssq5[:, 0:G],
                        sq_full[:].rearrange("p (a b) -> p a b", a=G),
                        mybir.AxisListType.X,
                        ADD,
                    )
                    nc.scalar.activation(
                        sq_k[:], kv_ps[:, 0:HD], AF.Square,
                        accum_out=ssq5[:, G : G + 1],
                    )
                    rms5 = small.tile([128, G + 1], F32, tag="rms5")
                    nc.scalar.activation(
                        rms5[:], ssq5[:], AF.Sqrt, bias=eps_sb[:], scale=1.0 / HD
                    )
                    nc.vector.reciprocal(rms5[:], rms5[:])

                    tr4 = pst.tile([128, G, 128], F32, tag="tr4")
                    trk = pst.tile([128, 128], F32, tag="trk")
                    for hc in range(G + 1):
                        if hc < G:
                            src = q_ps[:, hc * HD : (hc + 1) * HD]
                            cos_t, sin_t = cq_sb[:, st, :], sq_sb[:, st, :]
                        else:
                            src = kv_ps[:, 0:HD]
                            cos_t, sin_t = ck_sb[:, st, :], sk_sb[:, st, :]
                        rms = rms5[:, hc : hc + 1]

                        # rotate-half view of src (PSUM): src[p, (f+64) % 128]
                        rot = bass.AP(
                            tensor=src.tensor,
                            offset=src.offset + 64,
                            ap=[src.ap[0], [-64, 2], [1, 64]],
                        )
                        t1 = rope.tile([128, HD], F32, tag="t1")
                        t2 = rope.tile([128, HD], F32, tag="t2")
                        # t1 = (src * rms) * cos ; t2 = (rot(src) * rms) * sin
                        nc.vector.scalar_tensor_tensor(t1[:], src, rms, cos_t, MUL, MUL)
                        nc.vector.scalar_tensor_tensor(
                            t2[:].rearrange("p (a b) -> p a b", a=2),
                            rot,
                            rms,
                            sin_t.rearrange("p (a b) -> p a b", a=2),
                            MUL,
                            MUL,
                        )
                        qr = rope.tile([128, HD], F32, tag="qr")
                        nc.gpsimd.tensor_add(qr[:], t1[:], t2[:])

                        tr_dst = tr4[:, hc, :] if hc < G else trk[:]
                        nc.tensor.transpose(tr_dst, qr[:], ident[:])

                    # batched eviction: 4 q-head transposes -> qt_all columns
                    nc.scalar.copy(qt_all[:, :, st * 128 : (st + 1) * 128], tr4[:])
                    nc.scalar.copy(kt_sb[:, st * 128 : (st + 1) * 128], trk[:])

            # wo prefetch: phase-1 pools are released; load now so phase 3
            # never waits on this DMA.
            wo_pool = ctx.enter_context(tc.tile_pool(name="wo_pool", bufs=1))
            wo_sb = wo_pool.tile([128, G, D], F16, tag="wo")
            for i in range(2):
                nc.sync.dma_start(
                    wo_sb[:, 2 * i : 2 * i + 2, :],
                    dram3(wo, D, 128 * D, 2, D)(2 * i * 128 * D),
                )

            # ---------------- Phase 2+3: attention + fused o-projection ----------------
            with ExitStack() as p2:
                p2.enter_context(nc.named_scope("p2_attn"))
                epool = p2.enter_context(tc.tile_pool(name="exp", bufs=2))
                dpool = p2.enter_context(tc.tile_pool(name="den", bufs=3))
                opool = p2.enter_context(tc.tile_pool(name="osb", bufs=2))
                ps_s = p2.enter_context(tc.tile_pool(name="ps_s", bufs=2, space="PSUM"))
                ps_av = p2.enter_context(tc.tile_pool(name="ps_av", bufs=2, space="PSUM"))
                ps_o = p2.enter_context(tc.tile_pool(name="ps_o", bufs=2, space="PSUM"))

                # deferred o-projection work (one instruction per closure)
                p3q = deque()
                p3state = {}
                evict_rot = [nc.vector.tensor_copy, nc.scalar.copy,
                             nc.vector.tensor_copy, nc.vector.tensor_copy]

                def p3_mm(st, dc, h):
                    def f():
                        if h == 0:
                            p3state[(st, dc)] = ps_o.tile(
                                [128, 512], F32, name=f"op{st}_{dc}", tag="op"
                            )
                        nc.tensor.matmul(
                            p3state[(st, dc)][:],
                            outt[h][st // 4][:, (st % 4) * 128 : (st % 4 + 1) * 128],
                            wo_sb[:, h, dc * 512 : (dc + 1) * 512],
                            start=(h == 0),
                            stop=(h == G - 1),
                        )
                    return f

                def p3_evict(st, dc):
                    def f():
                        if dc == 0:
                            p3state[("o", st)] = opool.tile(
                                [128, 4, 512], F32, name=f"o_sb{st}", tag="o_sb"
                            )
                        op_ps = p3state.pop((st, dc))
                        evict_rot[dc](p3state[("o", st)][:, dc, :], op_ps[:])
                    return f

                def p3_dma(st):
                    def f():
                        o_sb = p3state.pop(("o", st))
                        nc.sync.dma_start(
                            bass.AP(
                                tensor=o.tensor,
                                offset=st * 128 * D,
                                ap=[[D, 128], [1, D]],
                            ),
                            o_sb[:].rearrange("p a b -> p (a b)"),
                        )
                    return f

                def push_p3(qc):
                    for st4 in range(4):
                        st = qc * 4 + st4
                        for dc in range(4):
                            for h in range(G):
                                p3q.append(p3_mm(st, dc, h))
                            p3q.append(p3_evict(st, dc))
                        p3q.append(p3_dma(st))

                def drain_p3(n):
                    for _ in range(n):
                        if not p3q:
                            return
                        p3q.popleft()()

                # deferred softmax tail: recip+norm of combo i-1 run early in
                # combo i, after the gpsimd partition_all_reduce has finished,
                # so they never head-of-line-block the DVE queue.
                pending_tail = [None]

                def flush_tail():
                    if pending_tail[0] is not None:
                        pending_tail[0]()
                        pending_tail[0] = None

                # AV matmuls run 3 batches behind their exp (spilling into the
                # next combo's stream) so PE never waits on the ACT pipeline
                # draining at a combo boundary.
                av_backlog = deque()

                def make_av(av_ps, exp_sb, kt):
                    def f():
                        nc.tensor.matmul(
                            av_ps[:],
                            v_sb[:, kt, :],
                            exp_sb[:, kt, :],
                            start=(kt == 0),
                            stop=(kt == KT - 1),
                        )
                    return f

                for qc in range(QC):
                    qsl = slice(qc * 512, (qc + 1) * 512)
                    for h in range(G):
                        cscope = nc.named_scope(f"c{qc}{h}")
                        cscope.__enter__()
                        exp_sb = epool.tile([128, KT, 512], F16, name="exp_sb", tag="exp")
                        av_ps = ps_av.tile([128, 512], F32, name="av_ps", tag="av")
                        acc = dpool.tile([128, 512], F16, name="acc", tag="acc")
                        for b in range(NB):
                            s_ps = ps_s.tile([128, 2, 512], F32, name="s_ps", tag="s")
                            for j in range(2):
                                kt = 2 * b + j
                                nc.tensor.matmul(
                                    s_ps[:, j, :],
                                    kt_sb[:, kt * 128 : (kt + 1) * 128],
                                    qt_all[:, h, qsl],
                                    start=True,
                                    stop=True,
                                )
                            nc.scalar.activation(
                                exp_sb[:, 2 * b : 2 * b + 2, :].rearrange(
                                    "p a b -> p (a b)"
                                ),
                                s_ps[:].rearrange("p a b -> p (a b)"),
                                AF.Exp,
                            )
                            # denominator partial-sum chain on DVE
                            for j in range(2):
                                kt = 2 * b + j
                                e = exp_sb[:, kt, :]
                                if kt == 1:
                                    nc.vector.tensor_add(acc[:], exp_sb[:, 0, :], e)
                                elif kt >= 2:
                                    nc.vector.tensor_add(acc[:], acc[:], e)
                            # AV accumulation from the lag-3 backlog
                            if len(av_backlog) >= 6:
                                av_backlog.popleft()()
                                av_backlog.popleft()()
                            if b == 6:
                                flush_tail()
                            if h >= 2:
                                drain_p3(6)
                            for j in range(2):
                                av_backlog.append(
                                    make_av(av_ps, exp_sb, 2 * b + j)
                                )
                        # denominator sum over key partitions, broadcast to all
                        den_bc = dpool.tile([128, 512], F32, name="den_bc", tag="den_bc")
                        nc.gpsimd.partition_all_reduce(
                            den_bc[:], acc[:], 128, bass_isa.ReduceOp.add
                        )

                        def make_tail(h=h, qc=qc, av_ps=av_ps, den_bc=den_bc):
                            def f():
                                with nc.named_scope(f"t{qc}{h}"):
                                    rbc = dpool.tile(
                                        [128, 512], F32, name="rbc", tag="rbc"
                                    )
                                    nc.vector.reciprocal_approx_fast(rbc[:], den_bc[:])
                                    nc.vector.tensor_mul(
                                        outt[h][qc][:], av_ps[:], rbc[:]
                                    )
                            return f

                        pending_tail[0] = make_tail()
                        cscope.__exit__(None, None, None)
                    push_p3(qc)
                while av_backlog:
                    av_backlog.popleft()()
                flush_tail()
                while p3q:
                    drain_p3(1)

    nc.compile()
    return nc


def _rope_tables(qw, kw):
    """Folded cos/sin tables. RoPE rotation with rotate-half; per-head RMS
    norm weight w and the attention scale sc are folded in:
      out[d] = qhat[d]*w[d]*cos[d]*sc + qhat[(d+64)%128]*(sgn)*w[(d+64)%128]*sin[d]*sc
    where sgn = -1 for d < 64 (rotate-half negates the upper half moved down).
    """
    inv_freq = 1.0 / (ROPE_BASE ** (np.arange(0, HD, 2, dtype=np.float32) / HD))
    t = np.arange(S, dtype=np.float32)
    freqs = np.outer(t, inv_freq).astype(np.float32)  # [S, 64]
    emb = np.concatenate([freqs, freqs], axis=1)  # [S, 128]
    cos = np.cos(emb).astype(np.float32)
    sin = np.sin(emb).astype(np.float32)

    sgn = np.where(np.arange(HD) < 64, np.float32(-1.0), np.float32(1.0))
    wshift_q = np.roll(qw, -64)  # w[(d+64)%128]
    wshift_k = np.roll(kw, -64)
    sc = np.float32(1.0 / np.sqrt(HD))
    cq = cos * qw[None, :] * sc
    sq_ = sin * (sgn * wshift_q)[None, :] * sc
    ck = cos * kw[None, :]
    sk_ = sin * (sgn * wshift_k)[None, :]
    return (
        np.ascontiguousarray(cq, dtype=np.float16),
        np.ascontiguousarray(sq_, dtype=np.float16),
        np.ascontiguousarray(ck, dtype=np.float16),
        np.ascontiguousarray(sk_, dtype=np.float16),
    )


def kernel(x, Wq, Wk, Wv, Wo, q_norm_w, k_norm_w):
    global _cached_nc, last_results
    x = np.asarray(x, dtype=np.float32)
    Wq = np.asarray(Wq, dtype=np.float32)
    Wk = np.asarray(Wk, dtype=np.float32)
    Wv = np.asarray(Wv, dtype=np.float32)
    Wo = np.asarray(Wo, dtype=np.float32)
    qw = np.asarray(q_norm_w, dtype=np.float32)
    kw = np.asarray(k_norm_w, dtype=np.float32)

    if _cached_nc is None:
        _cached_nc = _build_program()
    nc = _cached_nc

    cq, sq_, ck, sk_ = _rope_tables(qw, kw)

    in_maps = []
    for c in range(8):
        b, g = divmod(c, 4)
        in_maps.append(
            {
                "xt": np.ascontiguousarray(x[b].T.astype(np.float16)),
                "wq": np.ascontiguousarray(
                    Wq[:, g * DQ : (g + 1) * DQ].astype(np.float16)
                ),
                "wkv": np.ascontiguousarray(
                    np.concatenate(
                        [
                            Wk[:, g * HD : (g + 1) * HD],
                            Wv[:, g * HD : (g + 1) * HD],
                        ],
                        axis=1,
                    ).astype(np.float16)
                ),
                "wo": np.ascontiguousarray(
                    Wo[g * DQ : (g + 1) * DQ, :].astype(np.float16)
                ),
                "cq": cq,
                "sq": sq_,
                "ck": ck,
                "sk": sk_,
            }
        )

    last_results = run_bass_kernel_spmd(nc, in_maps, core_ids=list(range(8)))

    out = np.zeros((B, S, D), dtype=np.float32)
    for c in range(8):
        b = c // 4
        out[b] += last_results.results[c]["o"]
    return out


# revision 28
# speedup vs baseline: 1.0357x; 1.0357x over previous
"""Grouped-Query Attention forward pass on 8 Trainium2 NeuronCores.

Sharding: 2-way data parallel over batch x 4-way tensor parallel over KV
head groups. Core c = 4*b + g handles batch b and KV group g (4 query
heads + 1 KV head). Each core computes a partial o-projection output
(its head group's contribution, full [S, D]); the host sums the 4
partials per batch.

All matmul operands are fp16 (full PE rate, fp32 PSUM accumulation).

Device kernel per core:
  Phase 1 (per 128-row seq tile): q/k/v projections (contraction over D
    via xT tiles loaded 4 seq-tiles at a time for long DMA lines), RMS
    stats via tensor_tensor_reduce (DVE, eps seeded into the reduction),
    one Sqrt (ACT) + reciprocal (DVE) for all 5 head chunks, fused
    (src*rms)*cos|sin via scalar_tensor_tensor on DVE, rope add on
    GpSimd, PE transpose (4 q-head transposes share one PSUM bank, one
    batched eviction) -> qT/kT fp16. v stays natural.
  Phase 2+3 fused (qc-outer, head-inner): per 512-query chunk and head,
    16 key tiles in 8 two-bank PSUM batches: scoresT = kT.T @ qT -> one
    exp per batch (ACT, fp16 out) -> AV accumulation (lagged 2 batches).
    Softmax denominator: DVE add chain over exp tiles, one gpsimd
    partition_all_reduce (sum over key partitions, broadcast to all
    partitions), reciprocal_approx_fast + normalize on DVE.
    o-projection matmuls for the previous query chunk are interleaved as
    PE filler (skipping the first head so the last head's normalize has
    landed); PSUM evictions rotate over DVE/ACT; one output DMA per seq
    tile.
"""

import sys

sys.path.insert(0, "/opt/trn_rl_repo")

from collections import deque

import numpy as np

import concourse.bass as bass
import concourse.tile as tile
from concourse import bacc, bass_isa, mybir
from concourse.bass_utils import run_bass_kernel_spmd
from concourse.masks import make_identity

F32 = mybir.dt.float32
F32R = mybir.dt.float32r
F16 = mybir.dt.float16
AF = mybir.ActivationFunctionType
MUL = mybir.AluOpType.mult
ADD = mybir.AluOpType.add

B = 2
S = 2048
D = 2048
NH = 16
NKV = 4
HD = 128
G = NH // NKV  # 4 query heads per KV head / per core
DQ = G * HD  # 512 query dims per core
EPS = 1e-6
ROPE_BASE = 10000.0

NT = S // 128  # 16 sequence tiles
ND = D // 128  # 16 contraction slices
QC = 4  # query chunks of 512
KT = S // 128  # 16 key tiles
NB = KT // 2  # 8 two-tile score batches per (head, qc)
XG = 4  # seq tiles per x-column DMA group

_cached_nc = None
last_results = None  # BassKernelResults of the most recent run (for test.py)


def _build_program():
    nc = bacc.Bacc("TRN2", target_bir_lowering=False, debug=False)

    xt = nc.dram_tensor("xt", [D, S], F16, kind="ExternalInput").ap()
    wq = nc.dram_tensor("wq", [D, DQ], F16, kind="ExternalInput").ap()
    wkv = nc.dram_tensor("wkv", [D, 2 * HD], F16, kind="ExternalInput").ap()
    wo = nc.dram_tensor("wo", [DQ, D], F16, kind="ExternalInput").ap()
    cq = nc.dram_tensor("cq", [S, HD], F16, kind="ExternalInput").ap()
    sq = nc.dram_tensor("sq", [S, HD], F16, kind="ExternalInput").ap()
    ck = nc.dram_tensor("ck", [S, HD], F16, kind="ExternalInput").ap()
    sk = nc.dram_tensor("sk", [S, HD], F16, kind="ExternalInput").ap()
    o = nc.dram_tensor("o", [S, D], F32, kind="ExternalOutput").ap()

    def dram3(t, pstep, bstep, nb, line):
        # [128 partitions, nb, line] view of a DRAM matrix
        return lambda off: bass.AP(
            tensor=t.tensor, offset=off, ap=[[pstep, 128], [bstep, nb], [1, line]]
        )

    with tile.TileContext(nc) as tc:
        from contextlib import ExitStack

        with ExitStack() as ctx:
            persist = ctx.enter_context(tc.tile_pool(name="persist", bufs=1))

            # persistent SBUF tensors (all fp16)
            qt_all = persist.tile([128, G, S], F16, tag="qt")
            kt_sb = persist.tile([128, S], F16, tag="kt")
            v_sb = persist.tile([128, KT, HD], F16, tag="v")
            # separate tiles per (head, query-chunk) so the o-projection's
            # reads never alias the normalize writes of other chunks
            outt = [
                [
                    persist.tile([128, 512], F16, name=f"outt{h}_{q}", tag=f"outt{h}_{q}")
                    for q in range(QC)
                ]
                for h in range(G)
            ]
            ident = persist.tile([128, 128], F32, tag="ident")
            make_identity(nc, ident[:])
            eps_sb = persist.tile([128, 1], F32, tag="eps")
            nc.vector.memset(eps_sb[:], EPS)

            # ---------------- Phase 1: projections + RMS + RoPE ----------------
            with ExitStack() as p1:
                p1.enter_context(nc.named_scope("p1_proj"))
                wpool = p1.enter_context(tc.tile_pool(name="w1", bufs=1))
                xpool = p1.enter_context(tc.tile_pool(name="xcol", bufs=2))
                rope = p1.enter_context(tc.tile_pool(name="rope", bufs=4))
                small = p1.enter_context(tc.tile_pool(name="small", bufs=4))
                ps1 = p1.enter_context(tc.tile_pool(name="ps1", bufs=2, space="PSUM"))
                pst = p1.enter_context(tc.tile_pool(name="pst", bufs=2, space="PSUM"))

                wq_sb = wpool.tile([128, ND, DQ], F16, tag="wq")
                wkv_sb = wpool.tile([128, ND, 2 * HD], F16, tag="wkv")
                # first x-column group before everything else, split across 8
                # queues, so the first projection matmul starts ASAP
                xcol0 = xpool.tile([128, ND, XG * 128], F16, name="xcol0", tag="xcol")
                for i in range(8):
                    nc.sync.dma_start(
                        xcol0[:, 2 * i : 2 * i + 2, :],
                        dram3(xt, S, 128 * S, 2, XG * 128)(2 * i * 128 * S),
                    )
                # per-ds-slice weight DMAs: small first transfers so the first
                # projection matmuls are not gated on one monolithic load
                for i in range(ND):
                    nc.sync.dma_start(
                        wq_sb[:, i : i + 1, :],
                        dram3(wq, DQ, 128 * DQ, 1, DQ)(i * 128 * DQ),
                    )
                for i in range(4):
                    nc.sync.dma_start(
                        wkv_sb[:, 4 * i : 4 * i + 4, :],
                        dram3(wkv, 2 * HD, 128 * 2 * HD, 4, 2 * HD)(4 * i * 128 * 2 * HD),
                    )

                cq_sb = wpool.tile([128, NT, HD], F16, tag="cq")
                sq_sb = wpool.tile([128, NT, HD], F16, tag="sq")
                ck_sb = wpool.tile([128, NT, HD], F16, tag="ck")
                sk_sb = wpool.tile([128, NT, HD], F16, tag="sk")
                for t, t_sb in ((cq, cq_sb), (sq, sq_sb), (ck, ck_sb), (sk, sk_sb)):
                    nc.sync.dma_start(t_sb[:], dram3(t, HD, 128 * HD, NT, HD)(0))

                xcol = None
                for st in range(NT):
                    if st == 0:
                        xcol = xcol0
                    elif st % XG == 0:
                        xcol = xpool.tile([128, ND, XG * 128], F16, name="xcol", tag="xcol")
                        # xt[d, s] slice s in [st*128, +XG*128): partition d%128
                        for i in range(4):
                            nc.sync.dma_start(
                                xcol[:, 4 * i : 4 * i + 4, :],
                                dram3(xt, S, 128 * S, 4, XG * 128)(
                                    4 * i * 128 * S + st * 128
                                ),
                            )
                    xsl = slice((st % XG) * 128, (st % XG) * 128 + 128)

                    q_ps = ps1.tile([128, DQ], F32, tag="q_ps")
                    kv_ps = ps1.tile([128, 2 * HD], F32, tag="kv_ps")
                    for ds in range(ND):
                        nc.tensor.matmul(
                            q_ps[:],
                            xcol[:, ds, xsl],
                            wq_sb[:, ds, :],
                            start=(ds == 0),
                            stop=(ds == ND - 1),
                        )
                    for ds in range(ND):
                        nc.tensor.matmul(
                            kv_ps[:],
                            xcol[:, ds, xsl],
                            wkv_sb[:, ds, :],
                            start=(ds == 0),
                            stop=(ds == ND - 1),
                        )

                    # v: straight copy to natural fp16 layout
                    nc.scalar.copy(v_sb[:, st, :], kv_ps[:, HD : 2 * HD])

                    # RMS stats for all 5 head chunks: one batched Square for
                    # the q heads (per-head reduce on DVE), Square+accum for k,
                    # then rms5 = 1/sqrt(sum/HD + eps) for all 5 at once.
                    ssq5 = small.tile([128, G + 1], F32, tag="ssq5")
                    sq_full = small.tile([128, DQ], F16, tag="sq_full")
                    sq_k = small.tile([128, HD], F16, tag="sq_k")
                    nc.scalar.activation(sq_full[:], q_ps[:], AF.Square)
                    nc.vector.tensor_reduce(
                        ssq5[:, 0:G],
                        sq_full[:].rearrange("p (a b) -> p a b", a=G),
                        mybir.AxisListType.X,
                        ADD,
                    )
                    nc.scalar.activation(
                        sq_k[:], kv_ps[:, 0:HD], AF.Square,
                        accum_out=ssq5[:, G : G + 1],
                    )
                    rms5 = small.tile([128, G + 1], F32, tag="rms5")
                    nc.scalar.activation(
                        rms5[:], ssq5[:], AF.Sqrt, bias=eps_sb[:], scale=1.0 / HD
                    )
                    nc.vector.reciprocal(rms5[:], rms5[:])

                    tr4 = pst.tile([128, G, 128], F32, tag="tr4")
                    trk = pst.tile([128, 128], F32, tag="trk")
                    for hc in range(G + 1):
                        if hc < G:
                            src = q_ps[:, hc * HD : (hc + 1) * HD]
                            cos_t, sin_t = cq_sb[:, st, :], sq_sb[:, st, :]
                        else:
                            src = kv_ps[:, 0:HD]
                            cos_t, sin_t = ck_sb[:, st, :], sk_sb[:, st, :]
                        rms = rms5[:, hc : hc + 1]

                        # rotate-half view of src (PSUM): src[p, (f+64) % 128]
                        rot = bass.AP(
                            tensor=src.tensor,
                            offset=src.offset + 64,
                            ap=[src.ap[0], [-64, 2], [1, 64]],
                        )
                        t1 = rope.tile([128, HD], F32, tag="t1")
                        t2 = rope.tile([128, HD], F32, tag="t2")
                        # t1 = (src * rms) * cos ; t2 = (rot(src) * rms) * sin
                        nc.vector.scalar_tensor_tensor(t1[:], src, rms, cos_t, MUL, MUL)
                        nc.vector.scalar_tensor_tensor(
                            t2[:].rearrange("p (a b) -> p a b", a=2),
                            rot,
                            rms,
                            sin_t.rearrange("p (a b) -> p a b", a=2),
                            MUL,
                            MUL,
                        )
                        qr = rope.tile([128, HD], F32, tag="qr")
                        nc.gpsimd.tensor_add(qr[:], t1[:], t2[:])

                        tr_dst = tr4[:, hc, :] if hc < G else trk[:]
                        nc.tensor.transpose(tr_dst, qr[:], ident[:])

                    # batched eviction: 4 q-head transposes -> qt_all columns
                    nc.scalar.copy(qt_all[:, :, st * 128 : (st + 1) * 128], tr4[:])
                    nc.scalar.copy(kt_sb[:, st * 128 : (st + 1) * 128], trk[:])

            # wo prefetch: phase-1 pools are released; load now so phase 3
            # never waits on this DMA.
            wo_pool = ctx.enter_context(tc.tile_pool(name="wo_pool", bufs=1))
            wo_sb = wo_pool.tile([128, G, D], F16, tag="wo")
            for i in range(2):
                nc.sync.dma_start(
                    wo_sb[:, 2 * i : 2 * i + 2, :],
                    dram3(wo, D, 128 * D, 2, D)(2 * i * 128 * D),
                )

            # ---------------- Phase 2+3: attention + fused o-projection ----------------
            with ExitStack() as p2:
                p2.enter_context(nc.named_scope("p2_attn"))
                epool = p2.enter_context(tc.tile_pool(name="exp", bufs=2))
                dpool = p2.enter_context(tc.tile_pool(name="den", bufs=3))
                opool = p2.enter_context(tc.tile_pool(name="osb", bufs=2))
                ps_s = p2.enter_context(tc.tile_pool(name="ps_s", bufs=2, space="PSUM"))
                ps_av = p2.enter_context(tc.tile_pool(name="ps_av", bufs=2, space="PSUM"))
                ps_o = p2.enter_context(tc.tile_pool(name="ps_o", bufs=2, space="PSUM"))

                # deferred o-projection work (one instruction per closure)
                p3q = deque()
                p3state = {}
                evict_rot = [nc.scalar.copy, nc.scalar.copy,
                             nc.scalar.copy, nc.scalar.copy]

                def p3_mm(st, dc, h):
                    def f():
                        if h == 0:
                            p3state[(st, dc)] = ps_o.tile(
                                [128, 512], F32, name=f"op{st}_{dc}", tag="op"
                            )
                        nc.tensor.matmul(
                            p3state[(st, dc)][:],
                            outt[h][st // 4][:, (st % 4) * 128 : (st % 4 + 1) * 128],
                            wo_sb[:, h, dc * 512 : (dc + 1) * 512],
                            start=(h == 0),
                            stop=(h == G - 1),
                        )
                    return f

                def p3_evict(st, dc):
                    def f():
                        if dc == 0:
                            p3state[("o", st)] = opool.tile(
                                [128, 4, 512], F32, name=f"o_sb{st}", tag="o_sb"
                            )
                        op_ps = p3state.pop((st, dc))
                        evict_rot[dc](p3state[("o", st)][:, dc, :], op_ps[:])
                    return f

                def p3_dma(st):
                    def f():
                        o_sb = p3state.pop(("o", st))
                        nc.sync.dma_start(
                            bass.AP(
                                tensor=o.tensor,
                                offset=st * 128 * D,
                                ap=[[D, 128], [1, D]],
                            ),
                            o_sb[:].rearrange("p a b -> p (a b)"),
                        )
                    return f

                def push_p3(qc):
                    for st4 in range(4):
                        st = qc * 4 + st4
                        for dc in range(4):
                            for h in range(G):
                                p3q.append(p3_mm(st, dc, h))
                            p3q.append(p3_evict(st, dc))
                        p3q.append(p3_dma(st))

                def drain_p3(n):
                    for _ in range(n):
                        if not p3q:
                            return
                        p3q.popleft()()

                # deferred softmax tail: recip+norm of combo i-1 run early in
                # combo i, after the gpsimd partition_all_reduce has finished,
                # so they never head-of-line-block the DVE queue.
                pending_tail = [None]

                def flush_tail():
                    if pending_tail[0] is not None:
                        pending_tail[0]()
                        pending_tail[0] = None

                # AV matmuls run 3 batches behind their exp (spilling into the
                # next combo's stream) so PE never waits on the ACT pipeline
                # draining at a combo boundary.
                av_backlog = deque()

                def make_av(av_ps, exp_sb, kt):
                    def f():
                        nc.tensor.matmul(
                            av_ps[:],
                            v_sb[:, kt, :],
                            exp_sb[:, kt, :],
                            start=(kt == 0),
                            stop=(kt == KT - 1),
                        )
                    return f

                for qc in range(QC):
                    qsl = slice(qc * 512, (qc + 1) * 512)
                    for h in range(G):
                        cscope = nc.named_scope(f"c{qc}{h}")
                        cscope.__enter__()
                        exp_sb = epool.tile([128, KT, 512], F16, name="exp_sb", tag="exp")
                        av_ps = ps_av.tile([128, 512], F32, name="av_ps", tag="av")
                        acc = dpool.tile([128, 512], F16, name="acc", tag="acc")
                        for b in range(NB):
                            s_ps = ps_s.tile([128, 2, 512], F32, name="s_ps", tag="s")
                            for j in range(2):
                                kt = 2 * b + j
                                nc.tensor.matmul(
                                    s_ps[:, j, :],
                                    kt_sb[:, kt * 128 : (kt + 1) * 128],
                                    qt_all[:, h, qsl],
                                    start=True,
                                    stop=True,
                                )
                            nc.scalar.activation(
                                exp_sb[:, 2 * b : 2 * b + 2, :].rearrange(
                                    "p a b -> p (a b)"
                                ),
                                s_ps[:].rearrange("p a b -> p (a b)"),
                                AF.Exp,
                            )
                            # denominator partial-sum chain on DVE
                            for j in range(2):
                                kt = 2 * b + j
                                e = exp_sb[:, kt, :]
                                if kt == 1:
                                    nc.vector.tensor_add(acc[:], exp_sb[:, 0, :], e)
                                elif kt >= 2:
                                    nc.vector.tensor_add(acc[:], acc[:], e)
                            # AV accumulation from the lag-3 backlog
                            if len(av_backlog) >= 6:
                                av_backlog.popleft()()
                                av_backlog.popleft()()
                            if b == 4:
                                flush_tail()
                            if h >= 2:
                                drain_p3(6)
                            for j in range(2):
                                av_backlog.append(
                                    make_av(av_ps, exp_sb, 2 * b + j)
                                )
                        # denominator sum over key partitions, broadcast to all
                        den_bc = dpool.tile([128, 512], F32, name="den_bc", tag="den_bc")
                        nc.gpsimd.partition_all_reduce(
                            den_bc[:], acc[:], 128, bass_isa.ReduceOp.add
                        )

                        def make_tail(h=h, qc=qc, av_ps=av_ps, den_bc=den_bc):
                            def f():
                                with nc.named_scope(f"t{qc}{h}"):
                                    rbc = dpool.tile(
                                        [128, 512], F32, name="rbc", tag="rbc"
                                    )
                                    nc.vector.reciprocal_approx_fast(rbc[:], den_bc[:])
                                    nc.vector.tensor_mul(
                                        outt[h][qc][:], av_ps[:], rbc[:]
                                    )
                            return f

                        pending_tail[0] = make_tail()
                        cscope.__exit__(None, None, None)
                    push_p3(qc)
                while av_backlog:
                    av_backlog.popleft()()
                flush_tail()
                while p3q:
                    drain_p3(1)

    nc.compile()
    return nc


def _rope_tables(qw, kw):
    """Folded cos/sin tables. RoPE rotation with rotate-half; per-head RMS
    norm weight w and the attention scale sc are folded in:
      out[d] = qhat[d]*w[d]*cos[d]*sc + qhat[(d+64)%128]*(sgn)*w[(d+64)%128]*sin[d]*sc
    where sgn = -1 for d < 64 (rotate-half negates the upper half moved down).
    """
    inv_freq = 1.0 / (ROPE_BASE ** (np.arange(0, HD, 2, dtype=np.float32) / HD))
    t = np.arange(S, dtype=np.float32)
    freqs = np.outer(t, inv_freq).astype(np.float32)  # [S, 64]
    emb = np.concatenate([freqs, freqs], axis=1)  # [S, 128]
    cos = np.cos(emb).astype(np.float32)
    sin = np.sin(emb).astype(np.float32)

    sgn = np.where(np.arange(HD) < 64, np.float32(-1.0), np.float32(1.0))
    wshift_q = np.roll(qw, -64)  # w[(d+64)%128]
    wshift_k = np.roll(kw, -64)
    sc = np.float32(1.0 / np.sqrt(HD))
    cq = cos * qw[None, :] * sc
    sq_ = sin * (sgn * wshift_q)[None, :] * sc
    ck = cos * kw[None, :]
    sk_ = sin * (sgn * wshift_k)[None, :]
    return (
        np.ascontiguousarray(cq, dtype=np.float16),
        np.ascontiguousarray(sq_, dtype=np.float16),
        np.ascontiguousarray(ck, dtype=np.float16),
        np.ascontiguousarray(sk_, dtype=np.float16),
    )


def kernel(x, Wq, Wk, Wv, Wo, q_norm_w, k_norm_w):
    global _cached_nc, last_results
    x = np.asarray(x, dtype=np.float32)
    Wq = np.asarray(Wq, dtype=np.float32)
    Wk = np.asarray(Wk, dtype=np.float32)
    Wv = np.asarray(Wv, dtype=np.float32)
    Wo = np.asarray(Wo, dtype=np.float32)
    qw = np.asarray(q_norm_w, dtype=np.float32)
    kw = np.asarray(k_norm_w, dtype=np.float32)

    if _cached_nc is None:
        _cached_nc = _build_program()
    nc = _cached_nc

    cq, sq_, ck, sk_ = _rope_tables(qw, kw)

    in_maps = []
    for c in range(8):
        b, g = divmod(c, 4)
        in_maps.append(
            {
                "xt": np.ascontiguousarray(x[b].T.astype(np.float16)),
                "wq": np.ascontiguousarray(
                    Wq[:, g * DQ : (g + 1) * DQ].astype(np.float16)
                ),
                "wkv": np.ascontiguousarray(
                    np.concatenate(
                        [
                            Wk[:, g * HD : (g + 1) * HD],
                            Wv[:, g * HD : (g + 1) * HD],
                        ],
                        axis=1,
                    ).astype(np.float16)
                ),
                "wo": np.ascontiguousarray(
                    Wo[g * DQ : (g + 1) * DQ, :].astype(np.float16)
                ),
                "cq": cq,
                "sq": sq_,
                "ck": ck,
                "sk": sk_,
            }
        )

    last_results = run_bass_kernel_spmd(nc, in_maps, core_ids=list(range(8)))

    out = np.zeros((B, S, D), dtype=np.float32)
    for c in range(8):
        b = c // 4
        out[b] += last_results.results[c]["o"]
    return out
